# revision 1
# baseline (speedup 1.0000x reference)
"""C2Q attention kernel for 8 TRN2 NeuronCores.

Math (per batch):
    u      = (o_q @ W.T + b) / sqrt(H)          [Tq, H]
    score  = o_c @ u.T                           [Tc, Tq]
    prob   = softmax_j(score masked at j>=q_len) [Tc, Tq]
    out    = (prob * (i < c_len)) @ o_q          [Tc, H]

Device layout choices (everything lands K-on-partitions with zero on-chip
transposes of activations):
    u computed as [o, j]  (lhsT = W.T[h, o] tile, rhs = o_qT[h, j])
    score computed TRANSPOSED e=[j, i] (lhsT = u[o, j-block], rhs = o_cT[o, i])
    exp via ACT with per-partition bias qb[j] in {0, -1e7}: masked -> exactly 0
    denominator d[1, i] = ones[j,1].T @ e  (matmul partition-reduce)
    1/d transposed to columns via K=1 matmuls, folded into context eviction
    context [i, h] = e[j, i-block].T @ o_q[j, h]   (natural output layout)
c_len row masking is applied host-side (those rows are zeroed, never read).
"""

import os
import sys

import numpy as np

if "/opt/trn_rl_repo" not in sys.path:
    sys.path.insert(0, "/opt/trn_rl_repo")

B, Tc, Tq, H = 32, 512, 512, 1024
N_CORES = 8
B_LOCAL = B // N_CORES
KT = H // 128  # contraction tiles over h (8)
OT = H // 128  # linear-output tiles over o (8)
JT = Tq // 128  # question-token tiles (4)
IT = Tc // 128  # context-token tiles (4)
HB = H // 512  # free-dim blocks for context matmul (2)
SCALE = 1.0 / 32.0  # 1/sqrt(H)
NEG = -1.0e7


def _build_program(b_local: int, use_f32r: bool = True):
    import concourse.bacc as bacc
    import concourse.mybir as mybir
    import concourse.tile as tile

    f32 = mybir.dt.float32
    # reduced-precision single-pass fp32 matmul format; every tensor feeding
    # an fp32r matmul must itself be typed fp32r end-to-end (BIR verifier)
    mdt = mybir.dt.float32r if use_f32r else mybir.dt.float32

    nc = bacc.Bacc("TRN2", debug=False)

    # Small per-partition constants ride as extra columns of the big slabs
    # (a standalone [128, few] DMA costs 128 descriptors for ~2KB and clogs
    # the DGE descriptor stream during the ramp):
    #   wt slab 0 cols 1024:1032 = bias/32 arranged [p, o_tile], col 1032 = 1.0
    #   oqT slab 7 cols 512:516  = exp-bias qb (0 / -1e7) arranged [p, j_tile]
    WTW = H + 16  # wt slab width (pad)
    QTW = Tq + 8  # oqT slab width (pad)
    CTW = Tc + 8  # ocT slab width (pad); slab 0 col Tc = ones
    f16 = mybir.dt.float16
    oqT_d = nc.declare_dram_parameter("oqT", [b_local, KT, 128, QTW], f16, isOutput=False)
    ocT_d = nc.declare_dram_parameter("ocT", [b_local, KT, 128, CTW], f16, isOutput=False)
    oqN_d = nc.declare_dram_parameter("oqN", [b_local, Tq, H], f16, isOutput=False)
    wt_d = nc.declare_dram_parameter("wt", [KT, 128, WTW], f16, isOutput=False)
    bias_d = nc.declare_dram_parameter("biasP", [128, OT], f32, isOutput=False)
    out_d = nc.declare_dram_parameter("out", [b_local, Tc, H], f32, isOutput=True)

    with tile.TileContext(nc) as tc:
        with (
            tc.tile_pool(name="const", bufs=1) as cpool,
            tc.tile_pool(name="inp", bufs=2) as ipool,
            tc.tile_pool(name="work", bufs=1) as wpool,
            tc.tile_pool(name="outp", bufs=3) as opool,
            tc.tile_pool(name="ps_u", bufs=2, space="PSUM") as ps_u,
            tc.tile_pool(name="ps_s", bufs=2, space="PSUM") as ps_s,
            tc.tile_pool(name="ps_c", bufs=3, space="PSUM") as ps_c,
            tc.tile_pool(name="ps_d", bufs=1, space="PSUM") as ps_d,
        ):
            ones_s = cpool.tile([1, 1], f32)
            nc.vector.memset(ones_s, 1.0)

            # W tiles: one tile per k so the first matmuls depend only on the
            # first slices; DMAs interleaved with batch-0 oqT below.
            wt_k = [cpool.tile([128, WTW], f16, tag=f"wt{k}", name=f"wt{k}") for k in range(KT)]
            biasP = cpool.tile([128, OT], f32)

            for b in range(b_local):
                # per-k tiles keep DMA->matmul deps fine-grained during ramp
                oqT_k = [ipool.tile([128, QTW], f16, tag=f"oqT{k}", name=f"oqT{k}_{b}") for k in range(KT)]
                ocT_k = [ipool.tile([128, CTW], f16, tag=f"ocT{k}", name=f"ocT{k}_{b}") for k in range(KT)]
                oqN = ipool.tile([128, JT, H], f16, tag="oqN")
                qb = oqT_k[KT - 1][:, Tq : Tq + JT]
                ones = ocT_k[0][:, Tc : Tc + 1]
                if b == 0:
                    # one tiny DMA (~0.65us of descriptor stream) ahead of the
                    # bulk: the first Linear evictions depend on it
                    nc.sync.dma_start(out=biasP, in_=bias_d[:, :])
                for k in range(KT):
                    if b == 0:
                        nc.sync.dma_start(out=wt_k[k], in_=wt_d[k])
                    nc.sync.dma_start(out=oqT_k[k], in_=oqT_d[b, k])
                for k in range(KT):
                    nc.sync.dma_start(out=ocT_k[k], in_=ocT_d[b, k])
                for j in range(JT):
                    nc.sync.dma_start(
                        out=oqN[:, j, :], in_=oqN_d[b, j * 128 : (j + 1) * 128, :]
                    )

                # ---- Linear: u[o, j] = W'@o_q.T + b'  (W', b' pre-scaled by
                # 1/32 on host). For batch 0 the contraction is split into
                # quarters so the PE has runnable matmuls as soon as each
                # ~1.5MB of wt/oqT has streamed in.
                u = wpool.tile([128, OT, Tq], f16, tag="u")
                for o in range(OT):
                    ups = ps_u.tile([128, Tq], f32, tag="ups")
                    for k in range(KT):
                        nc.tensor.matmul(
                            ups,
                            wt_k[k][:, o * 128 : (o + 1) * 128],
                            oqT_k[k][:, :Tq],
                            start=(k == 0),
                            stop=(k == KT - 1),
                        )
                    nc.vector.tensor_scalar(
                        out=u[:, o, :],
                        in0=ups,
                        scalar1=biasP[:, o : o + 1],
                        scalar2=None,
                        op0=mybir.AluOpType.add,
                    )

                # ---- score_T + exp: e[j, i] = exp((u.T @ o_cT)/32 + qbias[j]),
                # with the denominator accumulation d[1, i] = sum_j e[j, i]
                # interleaved one step behind so its chain latency hides ----
                dps = ps_d.tile([1, Tc], f32, tag="dmisc", name=f"dps_{b}")
                e_tiles = []
                for jt in range(JT):
                    sps = ps_s.tile([128, Tc], f32, tag="sps")
                    for o in range(OT):
                        nc.tensor.matmul(
                            sps,
                            u[:, o, jt * 128 : (jt + 1) * 128],
                            ocT_k[o][:, :Tc],
                            start=(o == 0),
                            stop=(o == OT - 1),
                        )
                    e = wpool.tile([128, Tc], f16, tag=f"e{jt}")
                    nc.scalar.activation(
                        out=e,
                        in_=sps,
                        func=mybir.ActivationFunctionType.Exp,
                        bias=qb[:, jt : jt + 1],
                        scale=SCALE,
                    )
                    e_tiles.append(e)
                    if jt >= 1:
                        nc.tensor.matmul(
                            dps,
                            ones,
                            e_tiles[jt - 1],
                            start=(jt == 1),
                            stop=False,
                            skip_group_check=True,
                        )
                nc.tensor.matmul(
                    dps,
                    ones,
                    e_tiles[JT - 1],
                    start=False,
                    stop=True,
                    skip_group_check=True,
                )

                osb_tiles = {}

                def ctx_group(it, hb):
                    if it not in osb_tiles:
                        osb_tiles[it] = opool.tile(
                            [128, H], f32, tag="osb", name=f"osb{it}_{b}"
                        )
                    cps = ps_c.tile([128, 512], f32, tag="cps", name=f"cps{it}{hb}_{b}")
                    for jt in range(JT):
                        nc.tensor.matmul(
                            cps,
                            e_tiles[jt][:, it * 128 : (it + 1) * 128],
                            oqN[:, jt, hb * 512 : (hb + 1) * 512],
                            start=(jt == 0),
                            stop=(jt == JT - 1),
                        )
                    return cps

                def ctx_evict(it, hb, cps, r):
                    osb = osb_tiles[it]
                    nc.vector.tensor_scalar(
                        out=osb[:, hb * 512 : (hb + 1) * 512],
                        in0=cps,
                        scalar1=r,
                        scalar2=None,
                        op0=mybir.AluOpType.mult,
                    )
                    nc.sync.dma_start(
                        out=out_d[
                            b, it * 128 : (it + 1) * 128, hb * 512 : (hb + 1) * 512
                        ],
                        in_=osb[:, hb * 512 : (hb + 1) * 512],
                    )

                # first ctx group runs while the d copy drains on DVE
                cps00 = ctx_group(0, 0)
                dsb = wpool.tile([1, Tc], f32, tag="dsb")
                nc.vector.tensor_copy(out=dsb, in_=dps)

                # transpose 1/d to per-partition columns via K=1 matmuls
                r_cols = []
                for it in range(IT):
                    dcps = ps_d.tile([128, 1], f32, tag="dmisc", name=f"dcps{it}_{b}")
                    nc.tensor.matmul(
                        dcps,
                        dsb[:, it * 128 : (it + 1) * 128],
                        ones_s[0:1, 0:1],
                        start=True,
                        stop=True,
                    )
                    r = wpool.tile([128, 1], f32, tag=f"r{it}")
                    nc.vector.reciprocal(out=r, in_=dcps)
                    r_cols.append(r)

                cps01 = ctx_group(0, 1)
                ctx_evict(0, 0, cps00, r_cols[0])
                ctx_evict(0, 1, cps01, r_cols[0])
                for it in range(1, IT):
                    for hb in range(HB):
                        cps = ctx_group(it, hb)
                        ctx_evict(it, hb, cps, r_cols[it])

    nc.compile()
    return nc


def _host_inputs(o_c, o_q, W, b, q_lengths):
    """Build the per-core input maps (host-side sharding + re-layout).

    Linear operands (W, o_qT) ship as fp16 (same PE rate, half the
    ramp-critical DMA bytes); the 1/sqrt(H) scale is applied later as the
    Exp activation's scale argument, so W keeps its natural fp16 range.
    """
    WTW, QTW, CTW = H + 16, Tq + 8, Tc + 8
    NEG16 = np.float16(-60000.0)  # exp(x - 60000) == 0 exactly in fp32
    wt_host = np.zeros((KT, 128, WTW), np.float16)
    wt_host[:, :, :H] = W.T.reshape(KT, 128, H)
    bias_host = np.ascontiguousarray(b.reshape(OT, 128).T)  # [128, o_tile] f32
    jidx = np.arange(JT)[None, :] * 128 + np.arange(128)[:, None]  # [128, JT]
    in_maps = []
    for c in range(N_CORES):
        sl = slice(c * B_LOCAL, (c + 1) * B_LOCAL)
        oq = np.ascontiguousarray(o_q[sl].astype(np.float16))
        ocT = np.zeros((B_LOCAL, KT, 128, CTW), np.float16)
        ocT[:, :, :, :Tc] = o_c[sl].transpose(0, 2, 1).reshape(B_LOCAL, KT, 128, Tc)
        ocT[:, 0, :, Tc] = 1.0  # ones column for the denominator matmul
        oqT = np.zeros((B_LOCAL, KT, 128, QTW), np.float16)
        oqT[:, :, :, :Tq] = (
            o_q[sl].transpose(0, 2, 1).reshape(B_LOCAL, KT, 128, Tq)
        )
        for lb in range(B_LOCAL):
            ql = int(q_lengths[c * B_LOCAL + lb])
            # qb (exp bias: 0 valid / -60000 masked) rides in the last slab
            oqT[lb, KT - 1, :, Tq : Tq + JT] = np.where(
                jidx < ql, np.float16(0.0), NEG16
            )
        in_maps.append(
            {"oqT": oqT, "ocT": ocT, "oqN": oq, "wt": wt_host, "biasP": bias_host}
        )
    return in_maps


def kernel(**inputs) -> np.ndarray:
    o_c = np.asarray(inputs["o_c"], dtype=np.float32)
    o_q = np.asarray(inputs["o_q"], dtype=np.float32)
    W = np.asarray(inputs["W"], dtype=np.float32)
    b = np.asarray(inputs["b"], dtype=np.float32)
    q_lengths = np.asarray(inputs["q_lengths"]).astype(np.int64)
    c_lengths = np.asarray(inputs["c_lengths"]).astype(np.int64)

    from concourse.bass_utils import run_bass_kernel_spmd

    in_maps = _host_inputs(o_c, o_q, W, b, q_lengths)
    nc = _build_program(B_LOCAL)

    trace = bool(int(os.environ.get("KERNEL_TRACE", "0")))
    res = run_bass_kernel_spmd(
        nc, in_maps, core_ids=list(range(N_CORES)), trace=trace
    )
    if trace:
        kernel.last_results = res

    out = np.zeros((B, Tc, H), dtype=np.float32)
    for c in range(N_CORES):
        dev = res.results[c]["out"]
        for lb in range(B_LOCAL):
            g = c * B_LOCAL + lb
            cl = int(c_lengths[g])
            out[g, :cl] = dev[lb, :cl]
    return out

